# revision 3
# baseline (speedup 1.0000x reference)
"""ACMConv (adaptive channel mixing GCN layer) on 8 Trainium2 NeuronCores.

Strategy (graph/data parallel, edges partitioned by destination):
- Host: add self loops, compute symmetric GCN norms, partition edges by
  destination core (node block of 12500), group each core's edges into
  128-destination windows, pad each window to a uniform number M of
  128-edge tiles (uniform across cores -> one SPMD graph).
- Device phase 1 (message aggregation): for each 128-edge tile, an
  indirect DMA gathers the 128 source rows of x; the vector engine builds
  sel[e, j] = norm_e * (dest_off_e == j) in one fused tensor_scalar; the
  tensor engine computes psum[j, :] += sel.T @ x_gathered, accumulating a
  whole window. The window result is transposed (PE) and written to a DRAM
  staging buffer sT[128, nodes] (s = normalized-adjacency @ x).
- Device phase 2 (projections + gating): per 128-node tile, with
  host-pretransposed x, compute s@W_low.T, s@W_high.T, x@W_high.T,
  x@W_id.T, x@W_gate.T (+ biases via K=1 matmuls, z-weighted for the
  aggregated terms), softmax gate, and combine.
Output rows are node-contiguous so the host just concatenates core shards.
"""

import numpy as np

import concourse.bass as bass
import concourse.bacc as bacc
import concourse.mybir as mybir
import concourse.tile as tile
from concourse.bass_utils import run_bass_kernel_spmd

N_NODES = 100000
D = 128
NCORES = 8
NPC = N_NODES // NCORES            # 12500 nodes per core
P = 128
NWIN = (NPC + P - 1) // P          # 98 destination windows per core
NPC_PAD = NWIN * P                 # 12544
BW = 14                            # windows per edge-data block
NB = NWIN // BW                    # 7 blocks
assert NB * BW == NWIN

F32 = mybir.dt.float32
I32 = mybir.dt.int32


# ---------------------------------------------------------------- host side


def _graph_prep(edge_index):
    """Partition edges by destination core; build per-core tile arrays."""
    ei = np.asarray(edge_index).astype(np.int64)
    loops = np.arange(N_NODES, dtype=np.int64)
    row = np.concatenate([ei[0], loops])
    col = np.concatenate([ei[1], loops])
    deg = np.bincount(row, minlength=N_NODES).astype(np.float64)
    dis = 1.0 / np.sqrt(np.maximum(deg, 1.0))
    norm = (dis[row] * dis[col]).astype(np.float32)
    z_all = np.bincount(col, weights=norm.astype(np.float64), minlength=N_NODES)
    z_all = z_all.astype(np.float32)

    core = col // NPC
    per_core = []
    maxcnt = 0
    for c in range(NCORES):
        msk = core == c
        r_c = row[msk].astype(np.int32)
        d_c = (col[msk] - c * NPC).astype(np.int64)
        n_c = norm[msk]
        win = d_c // P
        order = np.argsort(win, kind="stable")
        r_c, d_c, n_c, win = r_c[order], d_c[order], n_c[order], win[order]
        cnt = np.bincount(win, minlength=NWIN)
        maxcnt = max(maxcnt, int(cnt.max()))
        per_core.append((r_c, d_c, n_c, win, cnt))
    M = (maxcnt + P - 1) // P
    cap = M * P

    eis, efs, zs = [], [], []
    for c in range(NCORES):
        r_c, d_c, n_c, win, cnt = per_core[c]
        starts = np.zeros(NWIN, np.int64)
        starts[1:] = np.cumsum(cnt)[:-1]
        slot = np.arange(len(win)) - starts[win]
        gidx = np.zeros((NWIN, cap), np.int32)
        offv = np.zeros((NWIN, cap), np.float32)
        nrmv = np.zeros((NWIN, cap), np.float32)
        gidx[win, slot] = r_c
        offv[win, slot] = (d_c % P).astype(np.float32)
        nrmv[win, slot] = n_c
        # device layout: [NB, 128, BW*M], tile column j = bw*M + m
        A = gidx.reshape(NB, BW, M, P).transpose(0, 3, 1, 2)
        ei_l = np.ascontiguousarray(A.reshape(NB, P, BW * M))
        O = offv.reshape(NB, BW, M, P).transpose(0, 3, 1, 2)
        Nn = nrmv.reshape(NB, BW, M, P).transpose(0, 3, 1, 2)
        ef_l = np.ascontiguousarray(
            np.stack([O, Nn], axis=-1).reshape(NB, P, BW * M * 2)
        )
        z_c = np.zeros((1, NPC_PAD), np.float32)
        z_c[0, :NPC] = z_all[c * NPC : (c + 1) * NPC]
        eis.append(ei_l)
        efs.append(ef_l)
        zs.append(z_c)
    return M, eis, efs, zs


# -------------------------------------------------------------- device graph

_GRAPH_CACHE = {}


def _build(M):
    if M in _GRAPH_CACHE:
        return _GRAPH_CACHE[M]
    nc = bacc.Bacc()
    x_ext = nc.declare_dram_parameter("x", [N_NODES, D], F32, isOutput=False)
    xT_ext = nc.declare_dram_parameter("xT", [D, NPC_PAD], F32, isOutput=False)
    ei_ext = nc.declare_dram_parameter("ei", [NB, P, BW * M], I32, isOutput=False)
    ef_ext = nc.declare_dram_parameter("ef", [NB, P, BW * M * 2], F32, isOutput=False)
    z_ext = nc.declare_dram_parameter("z", [1, NPC_PAD], F32, isOutput=False)
    w_ext = nc.declare_dram_parameter("wmat", [P, 3 * D + 3], F32, isOutput=False)
    b_ext = nc.declare_dram_parameter("bvec", [1, 3 * D + 3], F32, isOutput=False)
    c_ext = nc.declare_dram_parameter("consts", [P, 2 * P], F32, isOutput=False)
    out_ext = nc.declare_dram_parameter("out", [NPC_PAD, D], F32, isOutput=True)
    stagT = nc.dram_tensor("stagT", [D, NPC_PAD], F32)

    AL = mybir.AluOpType
    with tile.TileContext(nc) as tc:
        with (
            tc.tile_pool(name="const", bufs=1) as constp,
            tc.tile_pool(name="eib", bufs=2) as eip,
            tc.tile_pool(name="efb", bufs=2) as efp,
            tc.tile_pool(name="xg", bufs=12) as xgp,
            tc.tile_pool(name="sel", bufs=12) as selp,
            tc.tile_pool(name="s1", bufs=3) as s1p,
            tc.tile_pool(name="sT1", bufs=3) as sT1p,
            tc.tile_pool(name="p2in", bufs=3) as p2inp,
            tc.tile_pool(name="gate", bufs=3) as gatep,
            tc.tile_pool(name="comb", bufs=3) as combp,
            tc.tile_pool(name="ps_acc", bufs=2, space="PSUM") as pp_acc,
            tc.tile_pool(name="ps_t", bufs=1, space="PSUM") as pp_t,
            tc.tile_pool(name="ps_mm", bufs=1, space="PSUM") as pp_mm,
            tc.tile_pool(name="ps_g", bufs=1, space="PSUM") as pp_g,
        ):
            cs = constp.tile([P, 2 * P], F32)
            nc.sync.dma_start(out=cs[:], in_=c_ext[:])
            iota = cs[:, 0:P]
            ident = cs[:, P : 2 * P]
            wm = constp.tile([P, 3 * D + 3], F32)
            nc.sync.dma_start(out=wm[:], in_=w_ext[:])
            WlT = wm[:, 0:D]
            WhT = wm[:, D : 2 * D]
            WiT = wm[:, 2 * D : 3 * D]
            WgT = wm[:, 3 * D : 3 * D + 3]
            bv = constp.tile([1, 3 * D + 3], F32)
            nc.sync.dma_start(out=bv[:], in_=b_ext[:])
            b_low = bv[:, 0:D]
            b_high = bv[:, D : 2 * D]
            b_id = bv[:, 2 * D : 3 * D]
            b_gate = bv[:, 3 * D : 3 * D + 3]
            ones = constp.tile([1, P], F32)
            nc.vector.memset(ones[:], 1.0)
            z_sb = constp.tile([1, NPC_PAD], F32)
            nc.sync.dma_start(out=z_sb[:], in_=z_ext[:])

            # ---- phase 1: windowed segment sum of norm * x[src]
            for nb in range(NB):
                ei_sb = eip.tile([P, BW * M], I32)
                nc.sync.dma_start(out=ei_sb[:], in_=ei_ext[nb])
                ef_sb = efp.tile([P, BW * M * 2], F32)
                nc.sync.dma_start(out=ef_sb[:], in_=ef_ext[nb])
                for bw in range(BW):
                    w = nb * BW + bw
                    ps = pp_acc.tile([P, P], F32, tag="ps")
                    for m in range(M):
                        j = bw * M + m
                        xg = xgp.tile([P, P], F32, tag="xg")
                        nc.gpsimd.indirect_dma_start(
                            out=xg[:],
                            out_offset=None,
                            in_=x_ext[:],
                            in_offset=bass.IndirectOffsetOnAxis(
                                ap=ei_sb[:, j : j + 1], axis=0
                            ),
                        )
                        sel = selp.tile([P, P], F32, tag="sel")
                        nc.vector.tensor_scalar(
                            out=sel[:],
                            in0=iota,
                            scalar1=ef_sb[:, 2 * j : 2 * j + 1],
                            scalar2=ef_sb[:, 2 * j + 1 : 2 * j + 2],
                            op0=AL.is_equal,
                            op1=AL.mult,
                        )
                        nc.tensor.matmul(
                            ps[:], lhsT=sel[:], rhs=xg[:],
                            start=(m == 0), stop=(m == M - 1),
                        )
                    s_sb = s1p.tile([P, P], F32, tag="s1")
                    nc.scalar.copy(s_sb[:], ps[:])
                    psT = pp_t.tile([P, P], F32, tag="psT")
                    nc.tensor.transpose(psT[:], s_sb[:], ident)
                    sT_sb = sT1p.tile([P, P], F32, tag="sT1")
                    nc.vector.tensor_copy(out=sT_sb[:], in_=psT[:])
                    nc.sync.dma_start(
                        out=stagT[:, w * P : (w + 1) * P], in_=sT_sb[:]
                    )

            # ---- phase 2: projections, gate, combine
            for t in range(NWIN):
                c0 = t * P
                xT_sb = p2inp.tile([P, P], F32, tag="xT")
                nc.sync.dma_start(out=xT_sb[:], in_=xT_ext[:, c0 : c0 + P])
                sT_sb2 = p2inp.tile([P, P], F32, tag="sT2")
                nc.sync.dma_start(out=sT_sb2[:], in_=stagT[:, c0 : c0 + P])
                zrow = z_sb[0:1, c0 : c0 + P]

                ps_low = pp_mm.tile([P, P], F32, tag="ps_low")
                nc.tensor.matmul(ps_low[:], lhsT=sT_sb2[:], rhs=WlT, start=True, stop=False)
                nc.tensor.matmul(ps_low[:], lhsT=zrow, rhs=b_low, start=False, stop=True)
                ps_hl = pp_mm.tile([P, P], F32, tag="ps_hl")
                nc.tensor.matmul(ps_hl[:], lhsT=sT_sb2[:], rhs=WhT, start=True, stop=False)
                nc.tensor.matmul(ps_hl[:], lhsT=zrow, rhs=b_high, start=False, stop=True)
                ps_high = pp_mm.tile([P, P], F32, tag="ps_high")
                nc.tensor.matmul(ps_high[:], lhsT=xT_sb[:], rhs=WhT, start=True, stop=False)
                nc.tensor.matmul(ps_high[:], lhsT=ones[:], rhs=b_high, start=False, stop=True)
                ps_id = pp_mm.tile([P, P], F32, tag="ps_id")
                nc.tensor.matmul(ps_id[:], lhsT=xT_sb[:], rhs=WiT, start=True, stop=False)
                nc.tensor.matmul(ps_id[:], lhsT=ones[:], rhs=b_id, start=False, stop=True)
                ps_gate = pp_g.tile([P, 3], F32, tag="ps_gate")
                nc.tensor.matmul(ps_gate[:], lhsT=xT_sb[:], rhs=WgT, start=True, stop=False)
                nc.tensor.matmul(ps_gate[:], lhsT=ones[:], rhs=b_gate, start=False, stop=True)

                eg = gatep.tile([P, 3], F32, tag="eg")
                nc.scalar.activation(eg[:], ps_gate[:], mybir.ActivationFunctionType.Exp)
                gs = gatep.tile([P, 1], F32, tag="gs")
                nc.vector.tensor_reduce(
                    out=gs[:], in_=eg[:], axis=mybir.AxisListType.X, op=AL.add
                )
                gr = gatep.tile([P, 1], F32, tag="gr")
                nc.vector.reciprocal(gr[:], gs[:])
                g = gatep.tile([P, 3], F32, tag="g")
                nc.vector.tensor_scalar(
                    out=g[:], in0=eg[:], scalar1=gr[:, 0:1], scalar2=None, op0=AL.mult
                )

                # out = g0*ps_low + g1*(ps_high - ps_hl) + g2*ps_id, distributed
                # so no op reads two PSUM inputs.
                u = combp.tile([P, P], F32, tag="u")
                nc.scalar.activation(
                    u[:], ps_low[:], mybir.ActivationFunctionType.Copy,
                    scale=g[:, 0:1],
                )
                v1 = combp.tile([P, P], F32, tag="v1")
                nc.scalar.activation(
                    v1[:], ps_high[:], mybir.ActivationFunctionType.Copy,
                    scale=g[:, 1:2],
                )
                v2 = combp.tile([P, P], F32, tag="v2")
                nc.vector.tensor_scalar(
                    out=v2[:], in0=ps_hl[:], scalar1=g[:, 1:2], scalar2=None,
                    op0=AL.mult,
                )
                w2 = combp.tile([P, P], F32, tag="w2")
                nc.scalar.activation(
                    w2[:], ps_id[:], mybir.ActivationFunctionType.Copy,
                    scale=g[:, 2:3],
                )
                o = combp.tile([P, P], F32, tag="o")
                nc.vector.tensor_tensor(out=o[:], in0=u[:], in1=v1[:], op=AL.add)
                nc.vector.tensor_tensor(out=o[:], in0=o[:], in1=v2[:], op=AL.subtract)
                nc.vector.tensor_tensor(out=o[:], in0=o[:], in1=w2[:], op=AL.add)
                nc.sync.dma_start(out=out_ext[c0 : c0 + P, :], in_=o[:])

    nc.compile()
    _GRAPH_CACHE[M] = nc
    return nc


# -------------------------------------------------------------------- entry


def kernel(x, edge_index, W_low, b_low, W_high, b_high, W_id, b_id, W_gate, b_gate):
    x = np.ascontiguousarray(np.asarray(x, dtype=np.float32))
    M, eis, efs, zs = _graph_prep(edge_index)
    nc = _build(M)

    wmat = np.ascontiguousarray(
        np.concatenate(
            [
                np.asarray(W_low, np.float32).T,
                np.asarray(W_high, np.float32).T,
                np.asarray(W_id, np.float32).T,
                np.asarray(W_gate, np.float32).T,
            ],
            axis=1,
        )
    )
    bvec = np.ascontiguousarray(
        np.concatenate(
            [
                np.asarray(b_low, np.float32),
                np.asarray(b_high, np.float32),
                np.asarray(b_id, np.float32),
                np.asarray(b_gate, np.float32),
            ]
        )[None, :]
    )
    iota = np.tile(np.arange(P, dtype=np.float32), (P, 1))
    ident = np.eye(P, dtype=np.float32)
    consts = np.ascontiguousarray(np.concatenate([iota, ident], axis=1))

    in_maps = []
    for c in range(NCORES):
        xp = np.zeros((NPC_PAD, D), np.float32)
        xp[:NPC] = x[c * NPC : (c + 1) * NPC]
        xT = np.ascontiguousarray(xp.T)
        in_maps.append(
            dict(
                x=x,
                xT=xT,
                ei=eis[c],
                ef=efs[c],
                z=zs[c],
                wmat=wmat,
                bvec=bvec,
                consts=consts,
            )
        )

    res = run_bass_kernel_spmd(nc, in_maps, list(range(NCORES)))
    out = np.concatenate(
        [res.results[c]["out"][:NPC] for c in range(NCORES)], axis=0
    )
    return out
